# revision 2
# baseline (speedup 1.0000x reference)
"""Local+global causal self-attention (GQA + RMSNorm + RoPE) on 8 TRN2 cores.

Sharding: 8-way head-parallel. Core c owns q-heads {2c, 2c+1} sharing kv-head
c//2. v2 design: natural [s, d] layout projections from a host-pretransposed
bf16 xT, RMS+RoPE as free-dim vector ops, XBAR DMA transposes to [d, s] for
QK (k stored twice along partitions so both q heads contract at matching base
partitions), chunked sparse attention with fused denominator row +
partition_broadcast, partition-packed y for a contraction-128 output
projection, bf16 output summed on the host. Phase-1 (proj/rope/transpose) is
software-pipelined into the attention stage loop.
"""

import sys

sys.path.insert(0, "/opt/trn_rl_repo")

import json
import re

import numpy as np

import bass_rust
import concourse.bass as bass
import concourse.mybir as mybir
import concourse.tile as tile
from concourse.alu_op_type import AluOpType
from concourse.vector_clock import ScopedClock

P = 128
S = 2048
DIM = 1024
H = 16
KVH = 4
HD = 64
LW = 256
GT = 64
ROPE_BASE = 10000.0
N_CORES = 8
NT = S // P  # 16 s-tiles / q-blocks
NG = NT // 2  # 8 groups of 2 s-tiles
DC = DIM // P  # 8 contraction chunks
F32 = mybir.dt.float32
BF16 = mybir.dt.bfloat16
AF = mybir.ActivationFunctionType
EPS = float(np.finfo(np.float32).eps)
SCALE = 1.0 / 8.0  # 1/sqrt(HD)
KVW = 136  # kv_nat row: [k(64) | v(64) | ones(1) | pad]


def _patched_drain_and_barrier(self, tick_clock, wait_clock):
    # This walrus build rejects >2 sem waits on a single Drain (TPB_CTRL).
    # Split the end-of-kernel waits across SP nops (<=1 wait each), then
    # drain bare. SP executes waits in program order, so the drain still
    # observes everything.
    gc = tick_clock.global_clock
    vals = [int(v) for v in re.findall(r"\d+", repr(gc))]
    for i, v in enumerate(vals):
        if v <= 0:
            continue
        sub = [0] * len(vals)
        sub[i] = v
        nop_inst = self.nc.sync.nop(nofuse=True)
        wait_clock.add_sem_waits(
            nop_inst.ins, ScopedClock({None: bass_rust.VectorClock(sub)})
        )
    self.nc.sync.drain()
    self.nc.all_engine_barrier()
    assert self.sems is not None
    popped = self.nc._tile_sem_poison_stack.pop()
    assert popped is self._sem_poison
    self.nc.clear_and_free_semaphores(list(self.sems.allocated().values()))
    self.nc.all_engine_barrier()


tile.TileContext._drain_and_barrier = _patched_drain_and_barrier

_MAXW = 1  # this walrus rejects >1 sync wait on one instruction


def _split_waits_json(raw: bytes) -> bytes:
    """Cap on_wait count per instruction; spill excess onto NoOps placed
    just before (same engine, executes its waits first in program order)."""
    m = json.loads(raw)
    ctr = 0
    for fn in m["functions"]:
        for bb in fn["blocks"]:
            out = []
            for ins in bb["instructions"]:
                si = ins.get("sync_info") or {}
                ow = si.get("on_wait") or []
                if len(ow) > _MAXW:
                    extra, keep = ow[:-_MAXW], ow[-_MAXW:]
                    for i in range(0, len(extra), _MAXW):
                        ctr += 1
                        out.append(
                            {
                                "debug": ins.get("debug", 0),
                                "engine": ins["engine"],
                                "ins": [],
                                "outs": [],
                                "name": f"I-wsp{ctr}",
                                "opcode": "NoOp",
                                "sync_info": {
                                    "on_update": [],
                                    "on_wait": extra[i : i + _MAXW],
                                },
                            }
                        )
                    si = dict(si)
                    si["on_wait"] = keep
                    ins = dict(ins)
                    ins["sync_info"] = si
                out.append(ins)
            bb["instructions"] = out
    return json.dumps(m).encode()


def _plan(i):
    """Chunks for q-block i as (slot, jb, kn, mask) with ps column layout
    slot*256 + h*128 + q.  mask in {None, 'band', 'tri', 'sp2'}."""
    if i == 0:
        return [(2, 0, P, "tri")]
    if i == 1:
        return [(3, 0, P, None), (2, 1, P, "tri")]
    if i == 2:
        return [(1, 0, P, "sp2"), (3, 1, P, None), (2, 2, P, "tri")]
    return [(0, 0, P, "glob"), (1, i - 2, P, "band"), (2, i, P, "tri"),
            (3, i - 1, P, None)]


def _exp_cols(i):
    # column range of ps/pt actually written by QK for block i
    return {0: (512, 768), 1: (512, 1024), 2: (256, 1024)}.get(i, (0, 1024))


MSLOT = {"band": 0, "tri": 1, "sp2": 2, "glob": 3}


def build_nc(mode=6):
    # mode: 1=proj/rope/xbar only, 2=+qk/exp, 3=+pv, 4=+bcast/div, 5=+oproj/evac, 6=full
    nc = bass.Bass()
    xt_d = nc.dram_tensor("xt", [8, P, DC, 256], BF16, kind="ExternalInput")
    wq_d = nc.dram_tensor("wq", [P, DC, P], BF16, kind="ExternalInput")
    wkv_d = nc.dram_tensor("wkv", [P, DC, P], BF16, kind="ExternalInput")
    wo_d = nc.dram_tensor("wo", [P, DIM], BF16, kind="ExternalInput")
    cc_d = nc.dram_tensor("cc", [P, NT, HD], BF16, kind="ExternalInput")
    ss_d = nc.dram_tensor("ss", [P, NT, HD], BF16, kind="ExternalInput")
    mk_d = nc.dram_tensor("mk", [P, 4, 2 * P], BF16, kind="ExternalInput")
    id_d = nc.dram_tensor("idn", [P, P], BF16, kind="ExternalInput")
    e2_d = nc.dram_tensor("e2", [1, 2 * P], BF16, kind="ExternalInput")
    out_d = nc.dram_tensor("out", [S, DIM], BF16, kind="ExternalOutput")

    with (
        nc.allow_low_precision(reason="bf16 compute, tolerance 2e-2"),
        tile.TileContext(nc) as tc,
    ):
        with tc.tile_pool(name="persist", bufs=1) as pp:
            xT = pp.tile([P, DC, S], BF16, tag="xT")
            wq = pp.tile([P, DC, P], BF16, tag="wq")
            wkv = pp.tile([P, DC, P], BF16, tag="wkv")
            wo = pp.tile([P, DIM], BF16, tag="wo")
            cc = pp.tile([P, NT, HD], BF16, tag="cc")
            ss = pp.tile([P, NT, HD], BF16, tag="ss")
            mk = pp.tile([P, 4, 2 * P], BF16, tag="mk")
            idn = pp.tile([P, P], BF16, tag="idn")
            e2 = pp.tile([1, 2 * P], BF16, tag="e2")
            eps = pp.tile([P, 1], F32, tag="eps")
            q_nat = pp.tile([P, NT, 2, P], BF16, tag="q_nat")  # [s,t,h,d|pad]
            kv_nat = pp.tile([P, NT, KVW], BF16, tag="kv_nat")  # k|v|1
            qT = pp.tile([P, 2, NT * P], BF16, tag="qT")  # [d|junk, h, s]
            kvT = pp.tile([P, NT, P], BF16, tag="kvT")  # [kd|vd, t, s]

            # ---- constant loads (SP queue); xt chunk 0 early ----
            nc.sync.dma_start(wq[:], wq_d[:])
            nc.sync.dma_start(wkv[:], wkv_d[:])
            nc.sync.dma_start(xT[:, :, 0:256], xt_d[0])
            nc.sync.dma_start(xT[:, :, 256:512], xt_d[1])
            nc.sync.dma_start(cc[:], cc_d[:])
            nc.sync.dma_start(ss[:], ss_d[:])
            nc.sync.dma_start(mk[:], mk_d[:])
            nc.sync.dma_start(idn[:], id_d[:])
            nc.sync.dma_start(e2[:], e2_d[:])
            nc.sync.dma_start(wo[:], wo_d[:])
            for ct in range(2, 8):
                nc.sync.dma_start(xT[:, :, ct * 256 : (ct + 1) * 256], xt_d[ct])
            nc.vector.memset(eps[:], EPS)
            nc.gpsimd.memset(kv_nat[:, :, 128:129], 1.0)

            with (
                tc.tile_pool(name="pjp", bufs=2, space="PSUM") as pjp,
                tc.tile_pool(name="pspA", bufs=1, space="PSUM") as pspA,
                tc.tile_pool(name="pspB", bufs=1, space="PSUM") as pspB,
                tc.tile_pool(name="pyp", bufs=2, space="PSUM") as pyp,
                tc.tile_pool(name="pop", bufs=2, space="PSUM") as pop,
                tc.tile_pool(name="sqp", bufs=2) as sqp,
                tc.tile_pool(name="rfp", bufs=2) as rfp,
                tc.tile_pool(name="qnp", bufs=2) as qnp,
                tc.tile_pool(name="ptp", bufs=3) as ptp,
                tc.tile_pool(name="dnp", bufs=3) as dnp,
                tc.tile_pool(name="pbp", bufs=3) as pbp,
                tc.tile_pool(name="ypp", bufs=2) as ypp,
                tc.tile_pool(name="osp", bufs=2) as osp,
            ):
                pj_t = {}

                def emit_proj(g):
                    t0 = 2 * g
                    pj = pjp.tile([P, 512], F32, tag="pj")
                    pj_t[g] = pj
                    # cols: [q_t0 | q_t1 | kv_t0 | kv_t1]; accumulate each
                    # region contiguously (interleaved groups corrupt a bank)
                    for tt in range(2):
                        for co, w in ((tt * P, wq), (256 + tt * P, wkv)):
                            for dc in range(DC):
                                nc.tensor.matmul(
                                    pj[:, co : co + P],
                                    xT[:, dc, (t0 + tt) * P : (t0 + tt + 1) * P],
                                    w[:, dc, :],
                                    start=(dc == 0),
                                    stop=(dc == DC - 1),
                                )

                def emit_rope(g):
                    t0 = 2 * g
                    pj = pj_t.pop(g)
                    # rms sums -> rf = 1/(2*sqrt(mean+eps)) via Dsqrt;
                    # cc/ss tables carry the extra factor 2.
                    sq = sqp.tile([P, 512], BF16, tag="sq")
                    nc.scalar.activation(sq[:], pj[:], AF.Square)
                    rfg = rfp.tile([P, 8], F32, tag="rfg")
                    sq4 = sq[:].rearrange("p (a d) -> p a d", d=HD)
                    nc.vector.tensor_reduce(
                        rfg[:, 0:4], sq4[:, 0:4, :], axis=mybir.AxisListType.X,
                        op=AluOpType.add,
                    )
                    kv4 = sq4.rearrange("p (t c) d -> p t c d", c=2)
                    nc.vector.tensor_reduce(
                        rfg[:, 4:6], kv4[:, 2:4, 0, :], axis=mybir.AxisListType.X,
                        op=AluOpType.add,
                    )
                    rfs = rfp.tile([P, 8], F32, tag="rfs")
                    nc.scalar.activation(
                        rfs[:, 0:6], rfg[:, 0:6], AF.Sqrt, bias=eps[:],
                        scale=1.0 / HD,
                    )
                    rfr = rfp.tile([P, 8], F32, tag="rfr")
                    nc.vector.reciprocal(rfr[:, 0:6], rfs[:, 0:6])

                    # normalize (x0.5): qn = pj_q * rf, kn = pj_k * rf
                    qn = qnp.tile([P, 256], BF16, tag="qn")
                    qn4 = qn[:].rearrange("p (t h d) -> p t h d", t=2, h=2)
                    pj_q = pj[:, 0:256].rearrange("p (t h d) -> p t h d", t=2, h=2)
                    nc.vector.tensor_tensor(
                        qn4,
                        pj_q,
                        rfr[:, 0:4].rearrange("p (t h o) -> p t h o", t=2, o=1)
                        .to_broadcast((P, 2, 2, HD)),
                        op=AluOpType.mult,
                    )
                    kn = qnp.tile([P, P], BF16, tag="kn")
                    kn2 = kn[:].rearrange("p (t d) -> p t d", t=2)
                    pj_kv = pj[:, 256:512].rearrange("p (t c d) -> p t c d", t=2, c=2)
                    nc.vector.tensor_tensor(
                        kn2,
                        pj_kv[:, :, 0, :],
                        rfr[:, 4:6].rearrange("p (t o) -> p t o", o=1)
                        .to_broadcast((P, 2, HD)),
                        op=AluOpType.mult,
                    )
                    # v passthrough (psum -> DVE)
                    nc.vector.tensor_copy(
                        kv_nat[:, t0 : t0 + 2, HD:P], pj_kv[:, :, 1, :]
                    )

                    # rope: out = qn*(2cc) + swap(qn)*(2ss)
                    ccg = cc[:, t0 : t0 + 2, :]  # [P, 2, 64]
                    ssg = ss[:, t0 : t0 + 2, :]
                    HF = HD // 2
                    t1q = qnp.tile([P, 256], BF16, tag="t1q")
                    t14 = t1q[:].rearrange("p (t h d) -> p t h d", t=2, h=2)
                    nc.gpsimd.tensor_tensor(
                        t14,
                        qn4,
                        ccg.rearrange("p t (o d) -> p t o d", o=1)
                        .to_broadcast((P, 2, 2, HD)),
                        op=AluOpType.mult,
                    )
                    t2q = qnp.tile([P, 256], BF16, tag="t2q")
                    t25 = t2q[:].rearrange("p (t h c f) -> p t h c f", t=2, h=2, c=2)
                    qn5 = qn4.rearrange("p t h (c f) -> p t h c f", c=2)
                    ss5 = ssg.rearrange("p t (o c f) -> p t o c f", o=1, c=2)
                    nc.gpsimd.tensor_tensor(
                        t25[:, :, :, 0, :],
                        qn5[:, :, :, 1, :],
                        ss5[:, :, :, 0, :].to_broadcast((P, 2, 2, HF)),
                        op=AluOpType.mult,
                    )
                    nc.gpsimd.tensor_tensor(
                        t25[:, :, :, 1, :],
                        qn5[:, :, :, 0, :],
                        ss5[:, :, :, 1, :].to_broadcast((P, 2, 2, HF)),
                        op=AluOpType.mult,
                    )
                    nc.gpsimd.tensor_tensor(
                        q_nat[:, t0 : t0 + 2, :, 0:HD],
                        t14,
                        t25.rearrange("p t h c f -> p t h (c f)"),
                        op=AluOpType.add,
                    )
                    # k rope
                    t1k = qnp.tile([P, P], BF16, tag="t1k")
                    t1k2 = t1k[:].rearrange("p (t d) -> p t d", t=2)
                    nc.gpsimd.tensor_tensor(t1k2, kn2, ccg, op=AluOpType.mult)
                    t2k = qnp.tile([P, P], BF16, tag="t2k")
                    t2k3 = t2k[:].rearrange("p (t c f) -> p t c f", t=2, c=2)
                    kn3 = kn2.rearrange("p t (c f) -> p t c f", c=2)
                    ss3 = ssg.rearrange("p t (c f) -> p t c f", c=2)
                    nc.gpsimd.tensor_tensor(
                        t2k3[:, :, 0, :], kn3[:, :, 1, :], ss3[:, :, 0, :],
                        op=AluOpType.mult,
                    )
                    nc.gpsimd.tensor_tensor(
                        t2k3[:, :, 1, :], kn3[:, :, 0, :], ss3[:, :, 1, :],
                        op=AluOpType.mult,
                    )
                    nc.gpsimd.tensor_tensor(
                        kv_nat[:, t0 : t0 + 2, 0:HD], t1k2,
                        t2k3.rearrange("p t c f -> p t (c f)"),
                        op=AluOpType.add,
                    )
                    dbg_dump(g, qn, kn, rfg, rfr, t1q, t2q, pj)

                def dbg_dump(g, qn, kn, rfg, rfr, t1q, t2q, pj=None):
                    if g != 0 or not int(__import__("os").environ.get("KV2_DBG", "0")):
                        return
                    nc.sync.dma_start(out_d[768:896, 0:256], qn[:])
                    nc.sync.dma_start(out_d[768:896, 256:384], kn[:])
                    dbg16 = pp.tile([P, 16], BF16, tag="dbg16")
                    nc.vector.tensor_copy(dbg16[:, 0:8], rfg[:, 0:8])
                    nc.vector.tensor_copy(dbg16[:, 8:16], rfr[:, 0:8])
                    nc.sync.dma_start(out_d[768:896, 384:400], dbg16[:])
                    nc.sync.dma_start(out_d[768:896, 512:768], t1q[:])
                    dbgpj = pp.tile([P, 512], BF16, tag="dbgpj")
                    nc.vector.tensor_copy(dbgpj[:], pj[:])
                    nc.sync.dma_start(out_d[896:1024, 256:768], dbgpj[:])
                    nc.sync.dma_start(out_d[1024:1152, 0:256], xT[:, 0, 0:256])
                    nc.sync.dma_start(out_d[1024:1152, 256:384], xT[:, 3, 0:128])
                    nc.sync.dma_start(out_d[1152:1280, 0:128], wq[:, 0, :])
                    nc.sync.dma_start(out_d[1152:1280, 128:256], wq[:, 3, :])
                    nc.sync.dma_start(out_d[896:1024, 0:256], t2q[:])

                def emit_xbar(g):
                    for tt in range(2):
                        t = 2 * g + tt
                        nc.sync.dma_start(
                            qT[:, 0, t * P : (t + 1) * P], q_nat[:, t, 0, :],
                            transpose=True,
                        )
                        nc.sync.dma_start(
                            qT[:, 1, t * P : (t + 1) * P], q_nat[:, t, 1, :],
                            transpose=True,
                        )
                    for tt in range(2):
                        t = 2 * g + tt
                        nc.sync.dma_start(
                            kvT[:, t, :], kv_nat[:, t, 0:P], transpose=True
                        )

                # ---------------- attention stage emitters -----------------
                ones64 = mk[0:1, 1, 0:HD]  # tri row 0 = all-ones [1, 64]
                ps_t = {}
                pt_t = {}
                py_t = {}
                dn_t = {}
                pb_t = {}
                ysb_t = {}
                yp_t = {}
                po_t = {}
                os_t = {}

                def emit_qk(i):
                    psA = pspA.tile([P, 512], F32, tag="psA")
                    psB = pspB.tile([P, 512], F32, tag="psB")
                    ps_t[i] = (psA, psB)
                    for slot, jb, kn, m in _plan(i):
                        tgt = psA if slot < 2 else psB
                        co = (slot % 2) * 256
                        for h in range(2):
                            nc.tensor.matmul(
                                tgt[0:kn, co + h * P : co + (h + 1) * P],
                                kvT[0:HD, jb, 0:kn],
                                qT[0:HD, h, i * P : (i + 1) * P],
                                start=True,
                                stop=True,
                            )

                def emit_exp_mask(i):
                    psA, psB = ps_t.pop(i)
                    lo, hi = _exp_cols(i)
                    pt = ptp.tile([P, 1024], BF16, tag="pt")
                    pt_t[i] = pt
                    if lo < 512:
                        nc.scalar.activation(
                            pt[:, lo:512], psA[:, lo:512], AF.Exp, scale=SCALE
                        )
                    if hi > 512:
                        nc.scalar.activation(
                            pt[:, 512:hi], psB[:, 0 : hi - 512], AF.Exp,
                            scale=SCALE,
                        )
                    for slot, jb, kn, m in _plan(i):
                        if m is None:
                            continue
                        co = slot * 256
                        pt3 = pt[:, co : co + 256].rearrange(
                            "p (h q) -> p h q", h=2
                        )
                        nc.gpsimd.tensor_tensor(
                            pt3,
                            pt3,
                            mk[:, MSLOT[m] : MSLOT[m] + 1, 0:P].to_broadcast(
                                (P, 2, P)
                            ),
                            op=AluOpType.mult,
                        )


                def emit_pv(i):
                    pt = pt_t.pop(i)
                    if i == 3 and int(__import__("os").environ.get("KV2_DBG", "0")):
                        nc.sync.dma_start(out_d[1280:1408, 0:1024], pt[:])
                    py = pyp.tile([P, 512], F32, tag="py")
                    py_t[i] = py
                    plan = _plan(i)
                    for ci, (slot, jb, kn, m) in enumerate(plan):
                        if m == "glob":
                            kn = GT  # keys 64:128 are bias-masked to ~0
                        nc.tensor.matmul(
                            py[0 : HD + 1, 0:256],
                            kv_nat[0:kn, jb, HD : P + 1],
                            pt[0:kn, slot * 256 : slot * 256 + 256],
                            start=(ci == 0),
                            stop=(ci == len(plan) - 1),
                        )
                    dn = dnp.tile([1, 256], BF16, tag="dn")
                    dn_t[i] = dn
                    nc.vector.reciprocal(dn[:], py[HD : HD + 1, 0:256])
                    ysb = ypp.tile([P, P], BF16, tag="ysb")
                    ysb_t[i] = ysb
                    nc.scalar.copy(ysb[0:HD, :], py[0:HD, 0:P])
                    nc.vector.tensor_copy(ysb[HD:P, :], py[0:HD, P:256])

                def emit_bcast(i):
                    # packed reciprocal-den broadcast [128, 128]: two
                    # accumulating selector matmuls (head halves)
                    py = py_t.pop(i)
                    dn = dn_t.pop(i)
                    nc.tensor.matmul(
                        py[0:P, 256:384], e2[0:1, 0:P], dn[0:1, 0:P],
                        start=True, stop=False,
                    )
                    nc.tensor.matmul(
                        py[0:P, 256:384], e2[0:1, P : 2 * P], dn[0:1, P:256],
                        start=False, stop=True,
                    )
                    pb_t[i] = py

                def emit_div(i):
                    pb = pb_t.pop(i)
                    ysb = ysb_t.pop(i)
                    yp = ypp.tile([P, P], BF16, tag="yp")
                    yp_t[i] = yp
                    nc.vector.tensor_tensor(
                        yp[:], ysb[:], pb[0:P, 256:384], op=AluOpType.mult
                    )
                    if i == 3 and int(__import__("os").environ.get("KV2_DBG", "0")):
                        d1 = pp.tile([P, 256], BF16, tag="dbgpy")
                        nc.vector.tensor_copy(d1[0:HD + 1, :], pb[0 : HD + 1, 0:256])
                        nc.sync.dma_start(out_d[1408:1536, 0:256], d1[:])
                        d2 = pp.tile([P, P], BF16, tag="dbgpb")
                        nc.vector.tensor_copy(d2[:], pb[0:P, 256:384])
                        nc.sync.dma_start(out_d[1536:1664, 0:128], d2[:])
                        nc.sync.dma_start(out_d[1536:1664, 128:256], ysb[:])
                        nc.sync.dma_start(out_d[1536:1664, 256:384], yp[:])

                def emit_oproj(i):
                    pos = []
                    for n2 in range(2):
                        po = pop.tile([P, 512], F32, tag="po")
                        pos.append(po)
                        nc.tensor.matmul(
                            po[:],
                            yp_t.pop(i) if n2 == 1 else yp_t[i],
                            wo[:, n2 * 512 : (n2 + 1) * 512],
                            start=True,
                            stop=True,
                        )
                    po_t[i] = pos

                def emit_evac(i):
                    po0, po1 = po_t.pop(i)
                    if i % 2 == 0:
                        osb = osp.tile([P, 2, DIM], BF16, tag="osb")
                        os_t[i // 2] = osb
                    else:
                        osb = os_t[i // 2]
                    half = osb[:, i % 2, :]
                    nc.vector.tensor_copy(half[:, 0:512], po0[:])
                    nc.scalar.copy(half[:, 512:1024], po1[:])

                def emit_store(j):
                    osb = os_t.pop(j)
                    nc.sync.dma_start(
                        out_d[j * 256 : (j + 1) * 256, :].rearrange(
                            "(b r) n -> r b n", b=2
                        ),
                        osb[:],
                    )

                # ---------------- pipelined emission ------------------------
                emit_proj(0)
                emit_proj(1)
                emit_rope(0)
                emit_xbar(0)
                emit_proj(2)
                emit_rope(1)
                emit_xbar(1)
                emit_proj(3)
                emit_rope(2)
                emit_xbar(2)
                emit_proj(4)

                for i in range(NT + 7):
                    if i < NT and mode >= 2:
                        emit_qk(i)
                        emit_exp_mask(i)
                    if i % 2 == 0:
                        g = i // 2 + 3
                        if g < NG:
                            emit_rope(g)
                            emit_xbar(g)
                        g2 = i // 2 + 5
                        if g2 < NG:
                            emit_proj(g2)
                    if 2 <= i < NT + 2 and mode >= 3:
                        emit_pv(i - 2)
                    if 3 <= i < NT + 3 and mode >= 4:
                        emit_bcast(i - 3)
                        emit_div(i - 3)
                    if 4 <= i < NT + 4 and mode >= 5:
                        emit_oproj(i - 4)
                        emit_evac(i - 4)
                    if 7 <= i and (i - 7) % 2 == 0 and (i - 7) // 2 < NT // 2 \
                            and mode == 6:
                        emit_store((i - 7) // 2)
                if mode == 7:
                    # debug: dump q_nat (rows 0:512) and kv_nat (rows 512:768)
                    for ch in range(4):
                        nc.sync.dma_start(
                            out_d[ch * P : (ch + 1) * P, :].rearrange(
                                "p (t h d) -> p t h d", t=4, h=2
                            ),
                            q_nat[:, ch * 4 : (ch + 1) * 4, :, :],
                        )
                    for ch in range(2):
                        nc.sync.dma_start(
                            out_d[512 + ch * P : 512 + (ch + 1) * P, 0:1024]
                            .rearrange("p (t w) -> p t w", t=8),
                            kv_nat[:, ch * 8 : (ch + 1) * 8, 0:P],
                        )
                if mode < 6:
                    # dummy store so the output exists
                    dummy = osp.tile([P, 2, DIM], BF16, tag="osb")
                    nc.vector.memset(dummy[:], 0.0)
                    for j in range(NT // 2):
                        nc.sync.dma_start(
                            out_d[j * 256 : (j + 1) * 256, :].rearrange(
                                "(b r) n -> r b n", b=2
                            ),
                            dummy[:],
                        )

    return nc


def _host_constants():
    inv_freq = 1.0 / (ROPE_BASE ** (np.arange(0, HD, 2, dtype=np.float64) / HD))
    s = np.arange(S, dtype=np.float64)
    freqs = s[:, None] * inv_freq[None, :]  # [S, 32]
    cos = np.cos(freqs).astype(np.float32)
    sin = np.sin(freqs).astype(np.float32)
    cc = np.concatenate([cos, cos], axis=1)  # [S, 64]
    ss = np.concatenate([sin, -sin], axis=1)
    # natural tiled layout [p, t, d]
    cc_t = np.ascontiguousarray(cc.reshape(NT, P, HD).transpose(1, 0, 2))
    ss_t = np.ascontiguousarray(ss.reshape(NT, P, HD).transpose(1, 0, 2))

    kk = np.arange(P)[:, None]
    qq = np.arange(P)[None, :]
    band = (kk >= qq + 1).astype(np.float32)
    tri = (kk <= qq).astype(np.float32)
    sp2 = ((kk < GT) | (kk >= qq + 1)).astype(np.float32)
    glob = (kk < GT).astype(np.float32) + 0 * qq
    mkk = np.stack([band, tri, sp2, glob], axis=1)  # [128, 4, 128]
    mk2 = np.concatenate([mkk, mkk], axis=2)  # [128, 4, 256] both heads
    idn = np.eye(P, dtype=np.float32)
    e2 = np.zeros((1, 2, P), dtype=np.float32)
    e2[0, 0, 0:HD] = 1.0
    e2[0, 1, HD:P] = 1.0
    return cc_t, ss_t, mk2, idn, e2


def _bf16(x):
    import ml_dtypes

    return np.ascontiguousarray(x).astype(ml_dtypes.bfloat16)


def kernel(x, Wq, Wk, Wv, Wo, profile=False):
    x = np.asarray(x, dtype=np.float32)
    Wq = np.asarray(Wq, dtype=np.float32)
    Wk = np.asarray(Wk, dtype=np.float32)
    Wv = np.asarray(Wv, dtype=np.float32)
    Wo = np.asarray(Wo, dtype=np.float32)
    bsz = x.shape[0]
    x2 = x.reshape(S, DIM)

    cc_t, ss_t, mk2, idn, e2 = _host_constants()
    # xT chunked [8, 128, 8, 256]: xt[ct, p, dc, j] = x[ct*256 + j, dc*128+p]
    xt = np.ascontiguousarray(
        x2.T.reshape(DC, P, 8, 256).transpose(2, 1, 0, 3)
    )

    cc_b, ss_b, mk_b, xt_b = _bf16(cc_t), _bf16(ss_t), _bf16(mk2), _bf16(xt)
    id_b = _bf16(idn)
    e2_b = _bf16(e2.reshape(1, 2 * P))

    in_maps = []
    for c in range(N_CORES):
        g = c // 2
        wq_c = _bf16(
            Wq[:, c * P : (c + 1) * P].reshape(DC, P, P).transpose(1, 0, 2)
        )
        wkv_c = _bf16(
            np.concatenate(
                [Wk[:, g * HD : (g + 1) * HD], Wv[:, g * HD : (g + 1) * HD]],
                axis=1,
            )
            .reshape(DC, P, P)
            .transpose(1, 0, 2)
        )
        wo_c = _bf16(Wo[c * P : (c + 1) * P, :])
        in_maps.append(
            {
                "xt": xt_b,
                "wq": wq_c,
                "wkv": wkv_c,
                "wo": wo_c,
                "cc": cc_b,
                "ss": ss_b,
                "mk": mk_b,
                "idn": id_b,
                "e2": e2_b,
            }
        )

    from concourse import bass_utils
    from concourse.bass_utils import run_bass_kernel_spmd

    nc = build_nc()
    _orig_json = nc.to_json_bytes
    nc.to_json_bytes = lambda: _split_waits_json(_orig_json())
    exec_ns = None
    if profile:
        bass_utils.upload_artifacts = lambda tmpdir: tmpdir  # no bucket here
        try:
            res = run_bass_kernel_spmd(nc, in_maps, list(range(N_CORES)), trace=True)
            exec_ns = res.exec_time_ns
        except Exception as e:
            print("profile path failed, falling back:", repr(e))
            res = run_bass_kernel_spmd(nc, in_maps, list(range(N_CORES)))
    else:
        res = run_bass_kernel_spmd(nc, in_maps, list(range(N_CORES)))

    out = np.zeros((S, DIM), dtype=np.float32)
    for c in range(N_CORES):
        out += res.results[c]["out"].astype(np.float32)
    out = out.reshape(bsz, S, DIM)
    if profile:
        return out, exec_ns, res
    return out


# revision 3
# speedup vs baseline: 1.0204x; 1.0204x over previous
"""Local+global causal self-attention (GQA + RMSNorm + RoPE) on 8 TRN2 cores.

Sharding: 8-way head-parallel. Core c owns q-heads {2c, 2c+1} sharing kv-head
c//2. v2 design: natural [s, d] layout projections from a host-pretransposed
bf16 xT, RMS+RoPE as free-dim vector ops, XBAR DMA transposes to [d, s] for
QK (k stored twice along partitions so both q heads contract at matching base
partitions), chunked sparse attention with fused denominator row +
partition_broadcast, partition-packed y for a contraction-128 output
projection, bf16 output summed on the host. Phase-1 (proj/rope/transpose) is
software-pipelined into the attention stage loop.
"""

import sys

sys.path.insert(0, "/opt/trn_rl_repo")

import json
import re

import numpy as np

import bass_rust
import concourse.bass as bass
import concourse.mybir as mybir
import concourse.tile as tile
from concourse.alu_op_type import AluOpType
from concourse.vector_clock import ScopedClock

P = 128
S = 2048
DIM = 1024
H = 16
KVH = 4
HD = 64
LW = 256
GT = 64
ROPE_BASE = 10000.0
N_CORES = 8
NT = S // P  # 16 s-tiles / q-blocks
NG = NT // 2  # 8 groups of 2 s-tiles
DC = DIM // P  # 8 contraction chunks
F32 = mybir.dt.float32
BF16 = mybir.dt.bfloat16
AF = mybir.ActivationFunctionType
EPS = float(np.finfo(np.float32).eps)
SCALE = 1.0 / 8.0  # 1/sqrt(HD)
KVW = 136  # kv_nat row: [k(64) | v(64) | ones(1) | pad]


def _patched_drain_and_barrier(self, tick_clock, wait_clock):
    # This walrus build rejects >2 sem waits on a single Drain (TPB_CTRL).
    # Split the end-of-kernel waits across SP nops (<=1 wait each), then
    # drain bare. SP executes waits in program order, so the drain still
    # observes everything.
    gc = tick_clock.global_clock
    vals = [int(v) for v in re.findall(r"\d+", repr(gc))]
    for i, v in enumerate(vals):
        if v <= 0:
            continue
        sub = [0] * len(vals)
        sub[i] = v
        nop_inst = self.nc.sync.nop(nofuse=True)
        wait_clock.add_sem_waits(
            nop_inst.ins, ScopedClock({None: bass_rust.VectorClock(sub)})
        )
    self.nc.sync.drain()
    self.nc.all_engine_barrier()
    assert self.sems is not None
    popped = self.nc._tile_sem_poison_stack.pop()
    assert popped is self._sem_poison
    self.nc.clear_and_free_semaphores(list(self.sems.allocated().values()))
    self.nc.all_engine_barrier()


tile.TileContext._drain_and_barrier = _patched_drain_and_barrier

_MAXW = 1  # this walrus rejects >1 sync wait on one instruction


def _split_waits_json(raw: bytes) -> bytes:
    """Cap on_wait count per instruction; spill excess onto NoOps placed
    just before (same engine, executes its waits first in program order)."""
    m = json.loads(raw)
    ctr = 0
    for fn in m["functions"]:
        for bb in fn["blocks"]:
            out = []
            for ins in bb["instructions"]:
                si = ins.get("sync_info") or {}
                ow = si.get("on_wait") or []
                if len(ow) > _MAXW:
                    extra, keep = ow[:-_MAXW], ow[-_MAXW:]
                    for i in range(0, len(extra), _MAXW):
                        ctr += 1
                        out.append(
                            {
                                "debug": ins.get("debug", 0),
                                "engine": ins["engine"],
                                "ins": [],
                                "outs": [],
                                "name": f"I-wsp{ctr}",
                                "opcode": "NoOp",
                                "sync_info": {
                                    "on_update": [],
                                    "on_wait": extra[i : i + _MAXW],
                                },
                            }
                        )
                    si = dict(si)
                    si["on_wait"] = keep
                    ins = dict(ins)
                    ins["sync_info"] = si
                out.append(ins)
            bb["instructions"] = out
    return json.dumps(m).encode()


def _plan(i):
    """Chunks for q-block i as (slot, jb, kn, mask) with ps column layout
    slot*256 + h*128 + q.  mask in {None, 'band', 'tri', 'sp2'}."""
    if i == 0:
        return [(2, 0, P, "tri")]
    if i == 1:
        return [(3, 0, P, None), (2, 1, P, "tri")]
    if i == 2:
        return [(1, 0, P, "sp2"), (3, 1, P, None), (2, 2, P, "tri")]
    return [(0, 0, P, "glob"), (1, i - 2, P, "band"), (2, i, P, "tri"),
            (3, i - 1, P, None)]


def _exp_cols(i):
    # column range of ps/pt actually written by QK for block i
    return {0: (512, 768), 1: (512, 1024), 2: (256, 1024)}.get(i, (0, 1024))


MSLOT = {"band": 0, "tri": 1, "sp2": 2, "glob": 3}


def build_nc(mode=6):
    # mode: 1=proj/rope/xbar only, 2=+qk/exp, 3=+pv, 4=+bcast/div, 5=+oproj/evac, 6=full
    nc = bass.Bass()
    xt_d = nc.dram_tensor("xt", [8, P, DC, 256], BF16, kind="ExternalInput")
    wq_d = nc.dram_tensor("wq", [P, DC, P], BF16, kind="ExternalInput")
    wkv_d = nc.dram_tensor("wkv", [P, DC, P], BF16, kind="ExternalInput")
    wo_d = nc.dram_tensor("wo", [P, DIM], BF16, kind="ExternalInput")
    cc_d = nc.dram_tensor("cc", [P, NT, HD], BF16, kind="ExternalInput")
    ss_d = nc.dram_tensor("ss", [P, NT, HD], BF16, kind="ExternalInput")
    mk_d = nc.dram_tensor("mk", [P, 4, 2 * P], BF16, kind="ExternalInput")
    id_d = nc.dram_tensor("idn", [P, P], BF16, kind="ExternalInput")
    e2_d = nc.dram_tensor("e2", [1, 2 * P], BF16, kind="ExternalInput")
    out_d = nc.dram_tensor("out", [S, DIM], BF16, kind="ExternalOutput")

    with (
        nc.allow_low_precision(reason="bf16 compute, tolerance 2e-2"),
        tile.TileContext(nc) as tc,
    ):
        with tc.tile_pool(name="persist", bufs=1) as pp:
            xT = pp.tile([P, DC, S], BF16, tag="xT")
            wq = pp.tile([P, DC, P], BF16, tag="wq")
            wkv = pp.tile([P, DC, P], BF16, tag="wkv")
            wo = pp.tile([P, DIM], BF16, tag="wo")
            cc = pp.tile([P, NT, HD], BF16, tag="cc")
            ss = pp.tile([P, NT, HD], BF16, tag="ss")
            mk = pp.tile([P, 4, 2 * P], BF16, tag="mk")
            idn = pp.tile([P, P], BF16, tag="idn")
            e2 = pp.tile([1, 2 * P], BF16, tag="e2")
            eps = pp.tile([P, 1], F32, tag="eps")
            q_nat = pp.tile([P, NT, 2, P], BF16, tag="q_nat")  # [s,t,h,d|pad]
            kv_nat = pp.tile([P, NT, KVW], BF16, tag="kv_nat")  # k|v|1
            qT = pp.tile([P, 2, NT * P], BF16, tag="qT")  # [d|junk, h, s]
            kvT = pp.tile([P, NT, P], BF16, tag="kvT")  # [kd|vd, t, s]

            # ---- constant loads (SP queue); xt chunk 0 early ----
            nc.sync.dma_start(wq[:], wq_d[:])
            nc.sync.dma_start(wkv[:], wkv_d[:])
            nc.sync.dma_start(xT[:, :, 0:256], xt_d[0])
            nc.sync.dma_start(xT[:, :, 256:512], xt_d[1])
            nc.sync.dma_start(cc[:], cc_d[:])
            nc.sync.dma_start(ss[:], ss_d[:])
            nc.sync.dma_start(mk[:], mk_d[:])
            nc.sync.dma_start(idn[:], id_d[:])
            nc.sync.dma_start(e2[:], e2_d[:])
            nc.sync.dma_start(wo[:], wo_d[:])
            for ct in range(2, 8):
                nc.sync.dma_start(xT[:, :, ct * 256 : (ct + 1) * 256], xt_d[ct])
            nc.vector.memset(eps[:], EPS)
            nc.gpsimd.memset(kv_nat[:, :, 128:129], 1.0)

            with (
                tc.tile_pool(name="pjp", bufs=2, space="PSUM") as pjp,
                tc.tile_pool(name="pspA", bufs=1, space="PSUM") as pspA,
                tc.tile_pool(name="pspB", bufs=1, space="PSUM") as pspB,
                tc.tile_pool(name="pyp", bufs=2, space="PSUM") as pyp,
                tc.tile_pool(name="pop", bufs=2, space="PSUM") as pop,
                tc.tile_pool(name="sqp", bufs=2) as sqp,
                tc.tile_pool(name="rfp", bufs=2) as rfp,
                tc.tile_pool(name="qnp", bufs=2) as qnp,
                tc.tile_pool(name="ptp", bufs=3) as ptp,
                tc.tile_pool(name="dnp", bufs=3) as dnp,
                tc.tile_pool(name="pbp", bufs=3) as pbp,
                tc.tile_pool(name="ypp", bufs=2) as ypp,
                tc.tile_pool(name="osp", bufs=2) as osp,
            ):
                pj_t = {}

                def emit_proj(g):
                    t0 = 2 * g
                    pj = pjp.tile([P, 512], F32, tag="pj")
                    pj_t[g] = pj
                    # cols: [q_t0 | q_t1 | kv_t0 | kv_t1]; accumulate each
                    # region contiguously (interleaved groups corrupt a bank)
                    for tt in range(2):
                        for co, w in ((tt * P, wq), (256 + tt * P, wkv)):
                            for dc in range(DC):
                                nc.tensor.matmul(
                                    pj[:, co : co + P],
                                    xT[:, dc, (t0 + tt) * P : (t0 + tt + 1) * P],
                                    w[:, dc, :],
                                    start=(dc == 0),
                                    stop=(dc == DC - 1),
                                )

                def emit_rope(g):
                    t0 = 2 * g
                    pj = pj_t.pop(g)
                    # rms sums -> rf = 1/(2*sqrt(mean+eps)) via Dsqrt;
                    # cc/ss tables carry the extra factor 2.
                    sq = sqp.tile([P, 512], BF16, tag="sq")
                    nc.scalar.activation(sq[:], pj[:], AF.Square)
                    rfg = rfp.tile([P, 8], F32, tag="rfg")
                    sq4 = sq[:].rearrange("p (a d) -> p a d", d=HD)
                    nc.vector.tensor_reduce(
                        rfg[:, 0:4], sq4[:, 0:4, :], axis=mybir.AxisListType.X,
                        op=AluOpType.add,
                    )
                    kv4 = sq4.rearrange("p (t c) d -> p t c d", c=2)
                    nc.vector.tensor_reduce(
                        rfg[:, 4:6], kv4[:, 2:4, 0, :], axis=mybir.AxisListType.X,
                        op=AluOpType.add,
                    )
                    rfs = rfp.tile([P, 8], F32, tag="rfs")
                    nc.scalar.activation(
                        rfs[:, 0:6], rfg[:, 0:6], AF.Sqrt, bias=eps[:],
                        scale=1.0 / HD,
                    )
                    rfr = rfp.tile([P, 8], F32, tag="rfr")
                    nc.vector.reciprocal(rfr[:, 0:6], rfs[:, 0:6])

                    # normalize (x0.5): qn = pj_q * rf, kn = pj_k * rf
                    qn = qnp.tile([P, 256], BF16, tag="qn")
                    qn4 = qn[:].rearrange("p (t h d) -> p t h d", t=2, h=2)
                    pj_q = pj[:, 0:256].rearrange("p (t h d) -> p t h d", t=2, h=2)
                    nc.vector.tensor_tensor(
                        qn4,
                        pj_q,
                        rfr[:, 0:4].rearrange("p (t h o) -> p t h o", t=2, o=1)
                        .to_broadcast((P, 2, 2, HD)),
                        op=AluOpType.mult,
                    )
                    kn = qnp.tile([P, P], BF16, tag="kn")
                    kn2 = kn[:].rearrange("p (t d) -> p t d", t=2)
                    pj_kv = pj[:, 256:512].rearrange("p (t c d) -> p t c d", t=2, c=2)
                    nc.vector.tensor_tensor(
                        kn2,
                        pj_kv[:, :, 0, :],
                        rfr[:, 4:6].rearrange("p (t o) -> p t o", o=1)
                        .to_broadcast((P, 2, HD)),
                        op=AluOpType.mult,
                    )
                    # v passthrough (psum -> DVE)
                    nc.vector.tensor_copy(
                        kv_nat[:, t0 : t0 + 2, HD:P], pj_kv[:, :, 1, :]
                    )

                    # rope: out = qn*(2cc) + swap(qn)*(2ss)
                    ccg = cc[:, t0 : t0 + 2, :]  # [P, 2, 64]
                    ssg = ss[:, t0 : t0 + 2, :]
                    HF = HD // 2
                    t1q = qnp.tile([P, 256], BF16, tag="t1q")
                    t14 = t1q[:].rearrange("p (t h d) -> p t h d", t=2, h=2)
                    nc.gpsimd.tensor_tensor(
                        t14,
                        qn4,
                        ccg.rearrange("p t (o d) -> p t o d", o=1)
                        .to_broadcast((P, 2, 2, HD)),
                        op=AluOpType.mult,
                    )
                    t2q = qnp.tile([P, 256], BF16, tag="t2q")
                    t25 = t2q[:].rearrange("p (t h c f) -> p t h c f", t=2, h=2, c=2)
                    qn5 = qn4.rearrange("p t h (c f) -> p t h c f", c=2)
                    ss5 = ssg.rearrange("p t (o c f) -> p t o c f", o=1, c=2)
                    nc.gpsimd.tensor_tensor(
                        t25[:, :, :, 0, :],
                        qn5[:, :, :, 1, :],
                        ss5[:, :, :, 0, :].to_broadcast((P, 2, 2, HF)),
                        op=AluOpType.mult,
                    )
                    nc.gpsimd.tensor_tensor(
                        t25[:, :, :, 1, :],
                        qn5[:, :, :, 0, :],
                        ss5[:, :, :, 1, :].to_broadcast((P, 2, 2, HF)),
                        op=AluOpType.mult,
                    )
                    nc.gpsimd.tensor_tensor(
                        q_nat[:, t0 : t0 + 2, :, 0:HD],
                        t14,
                        t25.rearrange("p t h c f -> p t h (c f)"),
                        op=AluOpType.add,
                    )
                    # k rope
                    t1k = qnp.tile([P, P], BF16, tag="t1k")
                    t1k2 = t1k[:].rearrange("p (t d) -> p t d", t=2)
                    nc.gpsimd.tensor_tensor(t1k2, kn2, ccg, op=AluOpType.mult)
                    t2k = qnp.tile([P, P], BF16, tag="t2k")
                    t2k3 = t2k[:].rearrange("p (t c f) -> p t c f", t=2, c=2)
                    kn3 = kn2.rearrange("p t (c f) -> p t c f", c=2)
                    ss3 = ssg.rearrange("p t (c f) -> p t c f", c=2)
                    nc.gpsimd.tensor_tensor(
                        t2k3[:, :, 0, :], kn3[:, :, 1, :], ss3[:, :, 0, :],
                        op=AluOpType.mult,
                    )
                    nc.gpsimd.tensor_tensor(
                        t2k3[:, :, 1, :], kn3[:, :, 0, :], ss3[:, :, 1, :],
                        op=AluOpType.mult,
                    )
                    nc.gpsimd.tensor_tensor(
                        kv_nat[:, t0 : t0 + 2, 0:HD], t1k2,
                        t2k3.rearrange("p t c f -> p t (c f)"),
                        op=AluOpType.add,
                    )
                    dbg_dump(g, qn, kn, rfg, rfr, t1q, t2q, pj)

                def dbg_dump(g, qn, kn, rfg, rfr, t1q, t2q, pj=None):
                    if g != 0 or not int(__import__("os").environ.get("KV2_DBG", "0")):
                        return
                    nc.sync.dma_start(out_d[768:896, 0:256], qn[:])
                    nc.sync.dma_start(out_d[768:896, 256:384], kn[:])
                    dbg16 = pp.tile([P, 16], BF16, tag="dbg16")
                    nc.vector.tensor_copy(dbg16[:, 0:8], rfg[:, 0:8])
                    nc.vector.tensor_copy(dbg16[:, 8:16], rfr[:, 0:8])
                    nc.sync.dma_start(out_d[768:896, 384:400], dbg16[:])
                    nc.sync.dma_start(out_d[768:896, 512:768], t1q[:])
                    dbgpj = pp.tile([P, 512], BF16, tag="dbgpj")
                    nc.vector.tensor_copy(dbgpj[:], pj[:])
                    nc.sync.dma_start(out_d[896:1024, 256:768], dbgpj[:])
                    nc.sync.dma_start(out_d[1024:1152, 0:256], xT[:, 0, 0:256])
                    nc.sync.dma_start(out_d[1024:1152, 256:384], xT[:, 3, 0:128])
                    nc.sync.dma_start(out_d[1152:1280, 0:128], wq[:, 0, :])
                    nc.sync.dma_start(out_d[1152:1280, 128:256], wq[:, 3, :])
                    nc.sync.dma_start(out_d[896:1024, 0:256], t2q[:])

                def emit_xbar(g):
                    for tt in range(2):
                        t = 2 * g + tt
                        nc.sync.dma_start(
                            qT[:, 0, t * P : (t + 1) * P], q_nat[:, t, 0, :],
                            transpose=True,
                        )
                        nc.sync.dma_start(
                            qT[:, 1, t * P : (t + 1) * P], q_nat[:, t, 1, :],
                            transpose=True,
                        )
                    for tt in range(2):
                        t = 2 * g + tt
                        nc.sync.dma_start(
                            kvT[:, t, :], kv_nat[:, t, 0:P], transpose=True
                        )

                # ---------------- attention stage emitters -----------------
                ones64 = mk[0:1, 1, 0:HD]  # tri row 0 = all-ones [1, 64]
                ps_t = {}
                pt_t = {}
                py_t = {}
                dn_t = {}
                pb_t = {}
                ysb_t = {}
                yp_t = {}
                po_t = {}
                os_t = {}

                def emit_qk(i):
                    psA = pspA.tile([P, 512], F32, tag="psA")
                    psB = pspB.tile([P, 512], F32, tag="psB")
                    ps_t[i] = (psA, psB)
                    for slot, jb, kn, m in _plan(i):
                        tgt = psA if slot < 2 else psB
                        co = (slot % 2) * 256
                        for h in range(2):
                            nc.tensor.matmul(
                                tgt[0:kn, co + h * P : co + (h + 1) * P],
                                kvT[0:HD, jb, 0:kn],
                                qT[0:HD, h, i * P : (i + 1) * P],
                                start=True,
                                stop=True,
                            )

                def emit_exp_mask(i):
                    psA, psB = ps_t.pop(i)
                    lo, hi = _exp_cols(i)
                    pt = ptp.tile([P, 1024], BF16, tag="pt")
                    pt_t[i] = pt
                    if lo < 512:
                        nc.scalar.activation(
                            pt[:, lo:512], psA[:, lo:512], AF.Exp, scale=SCALE
                        )
                    if hi > 512:
                        nc.scalar.activation(
                            pt[:, 512:hi], psB[:, 0 : hi - 512], AF.Exp,
                            scale=SCALE,
                        )
                    for slot, jb, kn, m in _plan(i):
                        if m is None:
                            continue
                        co = slot * 256
                        pt3 = pt[:, co : co + 256].rearrange(
                            "p (h q) -> p h q", h=2
                        )
                        (nc.gpsimd if (i + slot) % 2 else nc.vector).tensor_tensor(
                            pt3,
                            pt3,
                            mk[:, MSLOT[m] : MSLOT[m] + 1, 0:P].to_broadcast(
                                (P, 2, P)
                            ),
                            op=AluOpType.mult,
                        )


                def emit_pv(i):
                    pt = pt_t.pop(i)
                    if i == 3 and int(__import__("os").environ.get("KV2_DBG", "0")):
                        nc.sync.dma_start(out_d[1280:1408, 0:1024], pt[:])
                    py = pyp.tile([P, 512], F32, tag="py")
                    py_t[i] = py
                    plan = _plan(i)
                    for ci, (slot, jb, kn, m) in enumerate(plan):
                        if m == "glob":
                            kn = GT  # keys 64:128 are bias-masked to ~0
                        nc.tensor.matmul(
                            py[0 : HD + 1, 0:256],
                            kv_nat[0:kn, jb, HD : P + 1],
                            pt[0:kn, slot * 256 : slot * 256 + 256],
                            start=(ci == 0),
                            stop=(ci == len(plan) - 1),
                        )
                    dn = dnp.tile([1, 256], BF16, tag="dn")
                    dn_t[i] = dn
                    nc.vector.reciprocal(dn[:], py[HD : HD + 1, 0:256])
                    ysb = ypp.tile([P, P], BF16, tag="ysb")
                    ysb_t[i] = ysb
                    nc.scalar.copy(ysb[0:HD, :], py[0:HD, 0:P])
                    nc.vector.tensor_copy(ysb[HD:P, :], py[0:HD, P:256])

                def emit_bcast(i):
                    # packed reciprocal-den broadcast [128, 128]: two
                    # accumulating selector matmuls (head halves)
                    py = py_t.pop(i)
                    dn = dn_t.pop(i)
                    nc.tensor.matmul(
                        py[0:P, 256:384], e2[0:1, 0:P], dn[0:1, 0:P],
                        start=True, stop=False,
                    )
                    nc.tensor.matmul(
                        py[0:P, 256:384], e2[0:1, P : 2 * P], dn[0:1, P:256],
                        start=False, stop=True,
                    )
                    pb_t[i] = py

                def emit_div(i):
                    pb = pb_t.pop(i)
                    ysb = ysb_t.pop(i)
                    yp = ypp.tile([P, P], BF16, tag="yp")
                    yp_t[i] = yp
                    nc.vector.tensor_tensor(
                        yp[:], ysb[:], pb[0:P, 256:384], op=AluOpType.mult
                    )
                    if i == 3 and int(__import__("os").environ.get("KV2_DBG", "0")):
                        d1 = pp.tile([P, 256], BF16, tag="dbgpy")
                        nc.vector.tensor_copy(d1[0:HD + 1, :], pb[0 : HD + 1, 0:256])
                        nc.sync.dma_start(out_d[1408:1536, 0:256], d1[:])
                        d2 = pp.tile([P, P], BF16, tag="dbgpb")
                        nc.vector.tensor_copy(d2[:], pb[0:P, 256:384])
                        nc.sync.dma_start(out_d[1536:1664, 0:128], d2[:])
                        nc.sync.dma_start(out_d[1536:1664, 128:256], ysb[:])
                        nc.sync.dma_start(out_d[1536:1664, 256:384], yp[:])

                def emit_oproj(i):
                    pos = []
                    for n2 in range(2):
                        po = pop.tile([P, 512], F32, tag="po")
                        pos.append(po)
                        nc.tensor.matmul(
                            po[:],
                            yp_t.pop(i) if n2 == 1 else yp_t[i],
                            wo[:, n2 * 512 : (n2 + 1) * 512],
                            start=True,
                            stop=True,
                        )
                    po_t[i] = pos

                def emit_evac(i):
                    po0, po1 = po_t.pop(i)
                    if i % 2 == 0:
                        osb = osp.tile([P, 2, DIM], BF16, tag="osb")
                        os_t[i // 2] = osb
                    else:
                        osb = os_t[i // 2]
                    half = osb[:, i % 2, :]
                    nc.vector.tensor_copy(half[:, 0:512], po0[:])
                    nc.scalar.copy(half[:, 512:1024], po1[:])

                def emit_store(j):
                    osb = os_t.pop(j)
                    nc.sync.dma_start(
                        out_d[j * 256 : (j + 1) * 256, :].rearrange(
                            "(b r) n -> r b n", b=2
                        ),
                        osb[:],
                    )

                # ---------------- pipelined emission ------------------------
                emit_proj(0)
                emit_proj(1)
                emit_rope(0)
                emit_xbar(0)
                emit_proj(2)
                emit_rope(1)
                emit_xbar(1)
                emit_proj(3)
                emit_rope(2)
                emit_xbar(2)
                emit_proj(4)

                for i in range(NT + 7):
                    if i < NT and mode >= 2:
                        emit_qk(i)
                        emit_exp_mask(i)
                    if i % 2 == 0:
                        g = i // 2 + 3
                        if g < NG:
                            emit_rope(g)
                            emit_xbar(g)
                        g2 = i // 2 + 5
                        if g2 < NG:
                            emit_proj(g2)
                    if 2 <= i < NT + 2 and mode >= 3:
                        emit_pv(i - 2)
                    if 3 <= i < NT + 3 and mode >= 4:
                        emit_bcast(i - 3)
                        emit_div(i - 3)
                    if 4 <= i < NT + 4 and mode >= 5:
                        emit_oproj(i - 4)
                        emit_evac(i - 4)
                    if 7 <= i and (i - 7) % 2 == 0 and (i - 7) // 2 < NT // 2 \
                            and mode == 6:
                        emit_store((i - 7) // 2)
                if mode == 7:
                    # debug: dump q_nat (rows 0:512) and kv_nat (rows 512:768)
                    for ch in range(4):
                        nc.sync.dma_start(
                            out_d[ch * P : (ch + 1) * P, :].rearrange(
                                "p (t h d) -> p t h d", t=4, h=2
                            ),
                            q_nat[:, ch * 4 : (ch + 1) * 4, :, :],
                        )
                    for ch in range(2):
                        nc.sync.dma_start(
                            out_d[512 + ch * P : 512 + (ch + 1) * P, 0:1024]
                            .rearrange("p (t w) -> p t w", t=8),
                            kv_nat[:, ch * 8 : (ch + 1) * 8, 0:P],
                        )
                if mode < 6:
                    # dummy store so the output exists
                    dummy = osp.tile([P, 2, DIM], BF16, tag="osb")
                    nc.vector.memset(dummy[:], 0.0)
                    for j in range(NT // 2):
                        nc.sync.dma_start(
                            out_d[j * 256 : (j + 1) * 256, :].rearrange(
                                "(b r) n -> r b n", b=2
                            ),
                            dummy[:],
                        )

    return nc


def _host_constants():
    inv_freq = 1.0 / (ROPE_BASE ** (np.arange(0, HD, 2, dtype=np.float64) / HD))
    s = np.arange(S, dtype=np.float64)
    freqs = s[:, None] * inv_freq[None, :]  # [S, 32]
    cos = np.cos(freqs).astype(np.float32)
    sin = np.sin(freqs).astype(np.float32)
    cc = np.concatenate([cos, cos], axis=1)  # [S, 64]
    ss = np.concatenate([sin, -sin], axis=1)
    # natural tiled layout [p, t, d]
    cc_t = np.ascontiguousarray(cc.reshape(NT, P, HD).transpose(1, 0, 2))
    ss_t = np.ascontiguousarray(ss.reshape(NT, P, HD).transpose(1, 0, 2))

    kk = np.arange(P)[:, None]
    qq = np.arange(P)[None, :]
    band = (kk >= qq + 1).astype(np.float32)
    tri = (kk <= qq).astype(np.float32)
    sp2 = ((kk < GT) | (kk >= qq + 1)).astype(np.float32)
    glob = (kk < GT).astype(np.float32) + 0 * qq
    mkk = np.stack([band, tri, sp2, glob], axis=1)  # [128, 4, 128]
    mk2 = np.concatenate([mkk, mkk], axis=2)  # [128, 4, 256] both heads
    idn = np.eye(P, dtype=np.float32)
    e2 = np.zeros((1, 2, P), dtype=np.float32)
    e2[0, 0, 0:HD] = 1.0
    e2[0, 1, HD:P] = 1.0
    return cc_t, ss_t, mk2, idn, e2


def _bf16(x):
    import ml_dtypes

    return np.ascontiguousarray(x).astype(ml_dtypes.bfloat16)


def kernel(x, Wq, Wk, Wv, Wo, profile=False):
    x = np.asarray(x, dtype=np.float32)
    Wq = np.asarray(Wq, dtype=np.float32)
    Wk = np.asarray(Wk, dtype=np.float32)
    Wv = np.asarray(Wv, dtype=np.float32)
    Wo = np.asarray(Wo, dtype=np.float32)
    bsz = x.shape[0]
    x2 = x.reshape(S, DIM)

    cc_t, ss_t, mk2, idn, e2 = _host_constants()
    # xT chunked [8, 128, 8, 256]: xt[ct, p, dc, j] = x[ct*256 + j, dc*128+p]
    xt = np.ascontiguousarray(
        x2.T.reshape(DC, P, 8, 256).transpose(2, 1, 0, 3)
    )

    cc_b, ss_b, mk_b, xt_b = _bf16(cc_t), _bf16(ss_t), _bf16(mk2), _bf16(xt)
    id_b = _bf16(idn)
    e2_b = _bf16(e2.reshape(1, 2 * P))

    in_maps = []
    for c in range(N_CORES):
        g = c // 2
        wq_c = _bf16(
            Wq[:, c * P : (c + 1) * P].reshape(DC, P, P).transpose(1, 0, 2)
        )
        wkv_c = _bf16(
            np.concatenate(
                [Wk[:, g * HD : (g + 1) * HD], Wv[:, g * HD : (g + 1) * HD]],
                axis=1,
            )
            .reshape(DC, P, P)
            .transpose(1, 0, 2)
        )
        wo_c = _bf16(Wo[c * P : (c + 1) * P, :])
        in_maps.append(
            {
                "xt": xt_b,
                "wq": wq_c,
                "wkv": wkv_c,
                "wo": wo_c,
                "cc": cc_b,
                "ss": ss_b,
                "mk": mk_b,
                "idn": id_b,
                "e2": e2_b,
            }
        )

    from concourse import bass_utils
    from concourse.bass_utils import run_bass_kernel_spmd

    nc = build_nc()
    _orig_json = nc.to_json_bytes
    nc.to_json_bytes = lambda: _split_waits_json(_orig_json())
    exec_ns = None
    if profile:
        bass_utils.upload_artifacts = lambda tmpdir: tmpdir  # no bucket here
        try:
            res = run_bass_kernel_spmd(nc, in_maps, list(range(N_CORES)), trace=True)
            exec_ns = res.exec_time_ns
        except Exception as e:
            print("profile path failed, falling back:", repr(e))
            res = run_bass_kernel_spmd(nc, in_maps, list(range(N_CORES)))
    else:
        res = run_bass_kernel_spmd(nc, in_maps, list(range(N_CORES)))

    out = np.zeros((S, DIM), dtype=np.float32)
    for c in range(N_CORES):
        out += res.results[c]["out"].astype(np.float32)
    out = out.reshape(bsz, S, DIM)
    if profile:
        return out, exec_ns, res
    return out
